# revision 22
# baseline (speedup 1.0000x reference)
"""DTVNet TV-prox cascade kernel for 8 Trainium2 NeuronCores.

Decomposition (hardcoded for image/sino of shape [2, 256, 256, 128] f32):
  - Data-parallel shard along D (axis 1): core k owns D slices
    [32k, 32k+32); each core receives a 44-slice chunk (6-slice halo on
    each side, zero-padded at the global edges) and computes its owned
    slices with zero communication. The dual variable p is masked to 0
    at out-of-domain slabs and at global d = 255, which makes the
    interior stencils exact at the sharding boundaries.
  - On-core layout: SBUF tiles [W=128 partitions, B, Dslab, Hcols].
    D/H finite differences are shifted-AP vector ops; the W-axis
    (partition) difference and its adjoint run on the TensorEngine as
    matmuls against constant bidiagonal 128x128 matrices, accumulating
    in PSUM. The W dual is tracked sign-flipped (s~ = -s) so both W
    matmuls use the same constant matrices every cascade.
  - H is processed in 5 overlapping chunks (halo 6) so all per-chunk
    state (t, ss, z, p, q, s~) fits in SBUF.
"""

import sys

import numpy as np

sys.path.insert(0, "/opt/trn_rl_repo")

_B, _D, _H, _W = 2, 256, 256, 128
_NCORES = 8
_DCH = _D // _NCORES          # 32 owned D slices per core
_HALO = 6
_ND = _DCH + 2 * _HALO        # 44 slabs incl ghosts
_HG = 54                      # owned H per chunk
_LAMB = 0.01
_CASC = 3
_MMG = 3                      # D slabs per matmul/PSUM group (2*3*66*4B < 2KB bank)

_RUNNER_CACHE = {}


def _h_chunks():
    out = []
    oh = 0
    while oh < _H:
        og = min(_HG, _H - oh)
        h0 = max(0, oh - _HALO)
        h1 = min(_H, oh + og + _HALO)
        out.append((h0, h1, oh - h0, oh - h0 + og, oh))
        oh += og
    return out


def _stencil_mats():
    # M1 ("Dw"): out[p] = z[p+1] - z[p] for p < 127, 0 at p = 127.
    m1 = np.zeros((128, 128), np.float32)
    for p in range(127):
        m1[p + 1, p] = 1.0
        m1[p, p] = -1.0
    # M2: adjoint contribution with s~ = -s: out[p] = s~[p] - s~[p-1].
    m2 = np.zeros((128, 128), np.float32)
    for p in range(128):
        m2[p, p] = 1.0
        if p >= 1:
            m2[p - 1, p] = -1.0
    return np.stack([m1, m2])


def _build_program(sigma, repeat=1):
    import contextlib

    from concourse import bacc, mybir
    from concourse.alu_op_type import AluOpType as OP
    from concourse.tile import TileContext

    f32 = mybir.dt.float32
    s0, s1, s2, s3 = [float(x) for x in sigma]
    nc = bacc.Bacc()
    # Host pre-transposes chunks to w-major [W, B, ND, H] so every DMA is a
    # natural <=3-dim pattern with contiguous final dim on both sides.
    img = nc.declare_dram_parameter("img", [_W, _B, _ND, _H], f32, isOutput=False)
    sin = nc.declare_dram_parameter("sino", [_W, _B, _ND, _H], f32, isOutput=False)
    mats = nc.declare_dram_parameter("mats", [2, 128, 128], f32, isOutput=False)
    maskp = nc.declare_dram_parameter("maskp", [128, _ND], f32, isOutput=False)
    outs = [
        nc.declare_dram_parameter(f"out{c}", [_W, _B, _DCH, _H], f32, isOutput=True)
        for c in range(_CASC)
    ]
    edge_ranges = [(0, _HALO), (_ND - _HALO - 1, _ND - 1)]

    with TileContext(nc) as tc:
        with (
            tc.tile_pool(name="const", bufs=1) as cpool,
            tc.tile_pool(name="tp", bufs=2) as tpool,
            tc.tile_pool(name="ssp", bufs=2) as sspool,
            tc.tile_pool(name="state", bufs=1) as stpool,
            tc.tile_pool(name="zp", bufs=1) as zpool,
            tc.tile_pool(name="ps", bufs=4, space="PSUM") as ppool,
        ):
            matsb = cpool.tile([128, 2, 128], f32)
            for mi in range(2):
                nc.sync.dma_start(out=matsb[:, mi, :], in_=mats[mi, :, :])
            msb = cpool.tile([128, _ND], f32)
            nc.sync.dma_start(out=msb[:], in_=maskp[:])

            rep_ctx = (
                tc.For_i(0, repeat, 1) if repeat > 1 else contextlib.nullcontext()
            )
            with rep_ctx:
              for (h0, h1, ow0, ow1, oh) in _h_chunks():
                F = h1 - h0
                og = ow1 - ow0
                last = h1 == _H
                t = tpool.tile([128, _B, _ND, F], f32, tag="t")
                ss = sspool.tile([128, _B, _ND, F], f32, tag="ss")
                nc.sync.dma_start(out=t[:], in_=img[:, :, :, h0:h1])
                nc.sync.dma_start(out=ss[:], in_=sin[:, :, :, h0:h1])
                nc.vector.tensor_scalar(ss[:], ss[:], _LAMB, None, OP.mult)
                z = zpool.tile([128, _B, _ND, F], f32, tag="z")
                p = stpool.tile([128, _B, _ND, F], f32, tag="p")
                q = stpool.tile([128, _B, _ND, F], f32, tag="q")
                st = stpool.tile([128, _B, _ND, F], f32, tag="st")
                # cascade 1 writes p/q/st directly (duals start at 0);
                # only p's last slab is never written and must be zero.
                nc.vector.memset(p[:, :, _ND - 1, :], 0.0)

                for c in range(_CASC):
                    # z = (1-lamb)*t + lamb*sino
                    # (z-scale on DVE: first toucher of the DMA'd t tile, so
                    # the multi-queue DMA waits land here once and later DVE
                    # ops inherit them through same-engine program order.)
                    nc.scalar.mul(z[:], t[:], 1.0 - _LAMB)
                    nc.vector.tensor_tensor(z[:], z[:], ss[:], OP.add)
                    w = t  # t consumed by z; reuse as scratch
                    # ---- p chain (D axis), slabs [0, ND-1) ----
                    nc.vector.tensor_tensor(
                        w[:, :, 0 : _ND - 1, :],
                        z[:, :, 0 : _ND - 1, :],
                        z[:, :, 1:_ND, :],
                        OP.subtract,
                    )
                    if c == 0:
                        nc.vector.tensor_scalar(
                            p[:, :, 0 : _ND - 1, :],
                            w[:, :, 0 : _ND - 1, :],
                            -s0,
                            s0,
                            OP.max,
                            OP.min,
                        )
                    else:
                        nc.gpsimd.tensor_tensor(
                            p[:, :, 0 : _ND - 1, :],
                            p[:, :, 0 : _ND - 1, :],
                            w[:, :, 0 : _ND - 1, :],
                            OP.add,
                        )
                        nc.vector.tensor_scalar(
                            p[:, :, 0 : _ND - 1, :],
                            p[:, :, 0 : _ND - 1, :],
                            -s0,
                            s0,
                            OP.max,
                            OP.min,
                        )
                    for (e0, e1) in edge_ranges:
                        n = e1 - e0
                        mb = (
                            msb[:, e0:e1]
                            .unsqueeze(1)
                            .unsqueeze(3)
                            .broadcast_to([128, _B, n, F])
                        )
                        nc.vector.tensor_tensor(
                            p[:, :, e0:e1, :], p[:, :, e0:e1, :], mb, OP.mult
                        )
                    # ---- q chain (H axis) ----
                    nc.vector.tensor_tensor(
                        w[:, :, :, 0 : F - 1],
                        z[:, :, :, 0 : F - 1],
                        z[:, :, :, 1:F],
                        OP.subtract,
                    )
                    if last:
                        nc.vector.memset(w[:, :, :, F - 1 : F], 0.0)
                    if c == 0:
                        nc.vector.tensor_scalar(
                            q[:], w[:], -s1, s1, OP.max, OP.min
                        )
                    else:
                        nc.gpsimd.tensor_tensor(q[:], q[:], w[:], OP.add)
                        nc.vector.tensor_scalar(
                            q[:], q[:], -s1, s1, OP.max, OP.min
                        )
                    # ---- s~ chain (W axis via TensorE) ----
                    for d0 in range(0, _ND, _MMG):
                        g = min(_MMG, _ND - d0)
                        ps = ppool.tile([128, _B, g, F], f32, tag="ps")
                        nc.tensor.matmul(
                            ps[:],
                            matsb[:, 0, :],
                            z[:, :, d0 : d0 + g, :],
                            start=True,
                            stop=True,
                        )
                        if c == 0:
                            nc.vector.tensor_scalar(
                                st[:, :, d0 : d0 + g, :],
                                ps[:],
                                -s2,
                                s2,
                                OP.max,
                                OP.min,
                            )
                        else:
                            nc.vector.tensor_tensor(
                                st[:, :, d0 : d0 + g, :],
                                st[:, :, d0 : d0 + g, :],
                                ps[:],
                                OP.add,
                            )
                    if c > 0:
                        nc.vector.tensor_scalar(
                            st[:], st[:], -s2, s2, OP.max, OP.min
                        )
                    # ---- znew (in place) ----
                    nc.vector.tensor_scalar(z[:], z[:], -s3, s3, OP.max, OP.min)
                    # ---- t update ----
                    nc.vector.tensor_tensor(
                        t[:, :, 1:_ND, :],
                        p[:, :, 0 : _ND - 1, :],
                        p[:, :, 1:_ND, :],
                        OP.subtract,
                    )
                    nc.vector.tensor_scalar(
                        t[:, :, 0:1, :], p[:, :, 0:1, :], -1.0, None, OP.mult
                    )
                    # zn - q computed on GPSIMD into the z buffer, in
                    # parallel with the DVE adjoint passes (keeps the slow
                    # engine off the strictly-ordered in-place t chain).
                    nc.gpsimd.tensor_tensor(z[:], z[:], q[:], OP.subtract)
                    nc.vector.tensor_tensor(
                        t[:, :, :, 1:F], t[:, :, :, 1:F], q[:, :, :, 0 : F - 1], OP.add
                    )
                    nc.vector.tensor_tensor(t[:], t[:], z[:], OP.add)
                    for d0 in range(0, _ND, _MMG):
                        g = min(_MMG, _ND - d0)
                        ps = ppool.tile([128, _B, g, F], f32, tag="ps")
                        nc.tensor.matmul(
                            ps[:],
                            matsb[:, 1, :],
                            st[:, :, d0 : d0 + g, :],
                            start=True,
                            stop=True,
                        )
                        nc.vector.tensor_tensor(
                            t[:, :, d0 : d0 + g, :],
                            t[:, :, d0 : d0 + g, :],
                            ps[:],
                            OP.add,
                        )
                    for b in range(_B):
                        nc.sync.dma_start(
                            out=outs[c][:, b, :, oh : oh + og],
                            in_=t[:, b, _HALO : _HALO + _DCH, ow0:ow1],
                        )
    nc.compile()
    return nc


def _make_runner(nc, n_cores):
    """Build a reusable (cached-jit) runner for the Bass program, modeled
    on concourse.bass2jax.run_bass_via_pjrt."""
    import jax
    from jax.experimental.shard_map import shard_map
    from jax.sharding import Mesh, PartitionSpec

    from concourse import bass2jax, mybir

    bass2jax.install_neuronx_cc_hook()

    partition_name = (
        nc.partition_id_tensor.name if nc.partition_id_tensor else None
    )
    in_names, out_names, out_avals = [], [], []
    for alloc in nc.m.functions[0].allocations:
        if not isinstance(alloc, mybir.MemoryLocationSet):
            continue
        name = alloc.memorylocations[0].name
        if alloc.kind == "ExternalInput":
            if name != partition_name:
                in_names.append(name)
        elif alloc.kind == "ExternalOutput":
            shape = tuple(alloc.tensor_shape)
            dtype = mybir.dt.np(alloc.dtype)
            out_names.append(name)
            out_avals.append(jax.core.ShapedArray(shape, dtype))
    n_params = len(in_names)
    n_outs = len(out_avals)
    all_in_names = tuple(in_names + out_names + ([partition_name] if partition_name else []))
    donate = tuple(range(n_params, n_params + n_outs))

    def _body(*args):
        operands = list(args)
        if partition_name is not None:
            operands.append(bass2jax.partition_id_tensor())
        return tuple(
            bass2jax._bass_exec_p.bind(
                *operands,
                out_avals=tuple(out_avals),
                in_names=all_in_names,
                out_names=tuple(out_names),
                lowering_input_output_aliases=(),
                sim_require_finite=True,
                sim_require_nnan=True,
                nc=nc,
            )
        )

    devices = jax.devices()[:n_cores]
    assert len(devices) == n_cores
    mesh = Mesh(np.asarray(devices), ("core",))
    in_specs = (PartitionSpec("core"),) * (n_params + n_outs)
    out_specs = (PartitionSpec("core"),) * n_outs
    sharded = jax.jit(
        shard_map(
            _body, mesh=mesh, in_specs=in_specs, out_specs=out_specs, check_rep=False
        ),
        donate_argnums=donate,
        keep_unused=True,
    )

    def _concat_inputs(in_maps):
        per_core = [[np.asarray(m[name]) for name in in_names] for m in in_maps]
        return [
            np.concatenate([per_core[c][i] for c in range(n_cores)], axis=0)
            for i in range(n_params)
        ]

    def run(in_maps):
        concat_in = _concat_inputs(in_maps)
        concat_zeros = [
            np.zeros((n_cores * a.shape[0], *a.shape[1:]), a.dtype) for a in out_avals
        ]
        out_arrs = sharded(*concat_in, *concat_zeros)
        return [
            {
                name: np.asarray(out_arrs[i]).reshape(
                    n_cores, *out_avals[i].shape
                )[c]
                for i, name in enumerate(out_names)
            }
            for c in range(n_cores)
        ]

    def time_device(in_maps, reps=20):
        """Device-exec wall time with inputs pre-staged on device and
        outputs left on device (no tunnel transfer in the timed region)."""
        import time as _time

        sharded_nodonate = jax.jit(
            shard_map(
                _body,
                mesh=mesh,
                in_specs=in_specs,
                out_specs=out_specs,
                check_rep=False,
            ),
            keep_unused=True,
        )
        from jax.sharding import NamedSharding

        concat_in = _concat_inputs(in_maps)
        concat_zeros = [
            np.zeros((n_cores * a.shape[0], *a.shape[1:]), a.dtype) for a in out_avals
        ]
        shard = NamedSharding(mesh, PartitionSpec("core"))
        dev_in = [jax.device_put(x, shard) for x in concat_in]
        dev_zero = [jax.device_put(x, shard) for x in concat_zeros]
        out = sharded_nodonate(*dev_in, *dev_zero)  # warm + compile
        jax.block_until_ready(out)
        times = []
        for _ in range(reps):
            t0 = _time.perf_counter()
            out = sharded_nodonate(*dev_in, *dev_zero)
            jax.block_until_ready(out)
            times.append(_time.perf_counter() - t0)
        return times

    run.time_device = time_device
    return run


def _get_runner(sigma):
    key = tuple(float(x) for x in np.asarray(sigma).ravel())
    if key not in _RUNNER_CACHE:
        nc = _build_program(sigma)
        _RUNNER_CACHE[key] = _make_runner(nc, _NCORES)
    return _RUNNER_CACHE[key]


def _build_in_maps(image, sino):
    from concurrent.futures import ThreadPoolExecutor

    mats = _stencil_mats()

    def one_core(k):
        d0 = k * _DCH - _HALO
        # w-major chunk [W, B, ND, H], ghost slabs zero
        img_c = np.zeros((_W, _B, _ND, _H), np.float32)
        sino_c = np.zeros((_W, _B, _ND, _H), np.float32)
        lo, hi = max(0, d0), min(_D, d0 + _ND)
        img_c[:, :, lo - d0 : hi - d0, :] = image[:, lo:hi].transpose(3, 0, 1, 2)
        sino_c[:, :, lo - d0 : hi - d0, :] = sino[:, lo:hi].transpose(3, 0, 1, 2)
        gd = d0 + np.arange(_ND)
        maskp = np.broadcast_to(
            ((gd >= 0) & (gd <= _D - 2)).astype(np.float32), (128, _ND)
        ).copy()
        return {"img": img_c, "sino": sino_c, "mats": mats, "maskp": maskp}

    with ThreadPoolExecutor(max_workers=_NCORES) as ex:
        return list(ex.map(one_core, range(_NCORES)))


def _reference_numpy(image, sino, sigma, nt):
    """Slow exact fallback for unexpected inputs (e.g. nt != 0)."""
    def fwd_diff(v, ax):
        d = np.diff(v, axis=ax)
        pad = [(0, 0)] * v.ndim
        pad[ax] = (0, 1)
        return np.pad(d, pad)

    def fwd_diff_t(pp, ax):
        n = pp.shape[ax]
        pad_front = [(0, 0)] * pp.ndim
        pad_front[ax] = (1, 0)
        a = np.pad(pp, pad_front)
        a = np.take(a, range(n), axis=ax)
        pad_back = [(0, 0)] * pp.ndim
        pad_back[ax] = (0, 1)
        b = np.pad(np.take(pp, range(n - 1), axis=ax), pad_back)
        return a - b

    t = image.astype(np.float32)
    out = [t]
    p = np.zeros_like(t)
    q = np.zeros_like(t)
    s = np.zeros_like(t)
    for c in range(_CASC):
        z = t - np.float32(_LAMB) * (t - sino)
        pn = np.clip(p - fwd_diff(z, 1), -sigma[0], sigma[0])
        qn = np.clip(q - fwd_diff(z, 2), -sigma[1], sigma[1])
        sn = np.clip(s - fwd_diff(z, 3), -sigma[2], sigma[2])
        zn = np.clip(z, -sigma[3], sigma[3])
        p = pn + nt[c] * (pn - p)
        q = qn + nt[c] * (qn - q)
        s = sn + nt[c] * (sn - s)
        t = fwd_diff_t(p, 1) + fwd_diff_t(q, 2) + fwd_diff_t(s, 3) + zn
        out.append(t.astype(np.float32))
    return tuple(out)


def kernel(image, sino, sigma, nt):
    image = np.asarray(image, np.float32)
    sino = np.asarray(sino, np.float32)
    sigma = np.asarray(sigma, np.float32)
    nt = np.asarray(nt, np.float32)

    if (
        image.shape != (_B, _D, _H, _W)
        or sino.shape != (_B, _D, _H, _W)
        or np.any(nt != 0.0)
    ):
        return _reference_numpy(image, sino, sigma, nt)

    try:
        runner = _get_runner(sigma)
        in_maps = _build_in_maps(image, sino)
        try:
            results = runner(in_maps)
        except Exception:
            results = runner(in_maps)  # one retry (transient device wedge)
    except Exception:
        return _reference_numpy(image, sino, sigma, nt)

    from concurrent.futures import ThreadPoolExecutor

    def gather(c):
        # per-core [W, B, DCH, H] -> concat d -> [B, D, H, W]
        cat = np.concatenate(
            [results[k][f"out{c}"] for k in range(_NCORES)], axis=2
        )
        return np.ascontiguousarray(cat.transpose(1, 2, 3, 0))

    with ThreadPoolExecutor(max_workers=_CASC) as ex:
        full = list(ex.map(gather, range(_CASC)))
    return (image, full[0], full[1], full[2])


# revision 24
# speedup vs baseline: 1.1154x; 1.1154x over previous
"""DTVNet TV-prox cascade kernel for 8 Trainium2 NeuronCores.

Decomposition (hardcoded for image/sino of shape [2, 256, 256, 128] f32):
  - Data-parallel shard along D (axis 1): core k owns D slices
    [32k, 32k+32); each core receives a 44-slice chunk (6-slice halo on
    each side, zero-padded at the global edges) and computes its owned
    slices with zero communication. The dual variable p is masked to 0
    at out-of-domain slabs and at global d = 255, which makes the
    interior stencils exact at the sharding boundaries.
  - On-core layout: SBUF tiles [W=128 partitions, B, Dslab, Hcols].
    D/H finite differences are shifted-AP vector ops; the W-axis
    (partition) difference and its adjoint run on the TensorEngine as
    matmuls against constant bidiagonal 128x128 matrices, accumulating
    in PSUM. The W dual is tracked sign-flipped (s~ = -s) so both W
    matmuls use the same constant matrices every cascade.
  - H is processed in 5 overlapping chunks (halo 6) so all per-chunk
    state (t, ss, z, p, q, s~) fits in SBUF.
"""

import sys

import numpy as np

sys.path.insert(0, "/opt/trn_rl_repo")

_B, _D, _H, _W = 2, 256, 256, 128
_NCORES = 8
_DCH = _D // _NCORES          # 32 owned D slices per core
_HALO = 6
_ND = _DCH + 2 * _HALO        # 44 slabs incl ghosts
_HG = 54                      # owned H per chunk
_LAMB = 0.01
_CASC = 3
_MMG = 3                      # D slabs per matmul/PSUM group (2*3*66*4B < 2KB bank)

_RUNNER_CACHE = {}


def _h_chunks():
    out = []
    oh = 0
    while oh < _H:
        og = min(_HG, _H - oh)
        h0 = max(0, oh - _HALO)
        h1 = min(_H, oh + og + _HALO)
        out.append((h0, h1, oh - h0, oh - h0 + og, oh))
        oh += og
    return out


def _stencil_mats():
    # M1 ("Dw"): out[p] = z[p+1] - z[p] for p < 127, 0 at p = 127.
    m1 = np.zeros((128, 128), np.float32)
    for p in range(127):
        m1[p + 1, p] = 1.0
        m1[p, p] = -1.0
    # M2: adjoint contribution with s~ = -s: out[p] = s~[p] - s~[p-1].
    m2 = np.zeros((128, 128), np.float32)
    for p in range(128):
        m2[p, p] = 1.0
        if p >= 1:
            m2[p - 1, p] = -1.0
    return np.stack([m1, m2])


def _build_program(sigma, repeat=1):
    import contextlib

    from concourse import bacc, mybir
    from concourse.alu_op_type import AluOpType as OP
    from concourse.tile import TileContext

    f32 = mybir.dt.float32
    s0, s1, s2, s3 = [float(x) for x in sigma]
    nc = bacc.Bacc()
    # Host pre-transposes chunks to w-major [W, B, ND, H] so every DMA is a
    # natural <=3-dim pattern with contiguous final dim on both sides.
    img = nc.declare_dram_parameter("img", [_W, _B, _ND, _H], f32, isOutput=False)
    sin = nc.declare_dram_parameter("sino", [_W, _B, _ND, _H], f32, isOutput=False)
    mats = nc.declare_dram_parameter("mats", [2, 128, 128], f32, isOutput=False)
    maskp = nc.declare_dram_parameter("maskp", [128, _ND], f32, isOutput=False)
    outs = [
        nc.declare_dram_parameter(f"out{c}", [_W, _B, _DCH, _H], f32, isOutput=True)
        for c in range(_CASC)
    ]
    edge_ranges = [(0, _HALO), (_ND - _HALO - 1, _ND - 1)]

    with TileContext(nc) as tc:
        with (
            tc.tile_pool(name="const", bufs=1) as cpool,
            tc.tile_pool(name="tp", bufs=2) as tpool,
            tc.tile_pool(name="ssp", bufs=2) as sspool,
            tc.tile_pool(name="state", bufs=1) as stpool,
            tc.tile_pool(name="zp", bufs=1) as zpool,
            tc.tile_pool(name="ps", bufs=8, space="PSUM") as ppool,
        ):
            matsb = cpool.tile([128, 2, 128], f32)
            for mi in range(2):
                nc.sync.dma_start(out=matsb[:, mi, :], in_=mats[mi, :, :])
            msb = cpool.tile([128, _ND], f32)
            nc.sync.dma_start(out=msb[:], in_=maskp[:])

            rep_ctx = (
                tc.For_i(0, repeat, 1) if repeat > 1 else contextlib.nullcontext()
            )
            with rep_ctx:
              for (h0, h1, ow0, ow1, oh) in _h_chunks():
                F = h1 - h0
                og = ow1 - ow0
                last = h1 == _H
                t = tpool.tile([128, _B, _ND, F], f32, tag="t")
                ss = sspool.tile([128, _B, _ND, F], f32, tag="ss")
                nc.sync.dma_start(out=t[:], in_=img[:, :, :, h0:h1])
                nc.sync.dma_start(out=ss[:], in_=sin[:, :, :, h0:h1])
                nc.vector.tensor_scalar(ss[:], ss[:], _LAMB, None, OP.mult)
                z = zpool.tile([128, _B, _ND, F], f32, tag="z")
                p = stpool.tile([128, _B, _ND, F], f32, tag="p")
                q = stpool.tile([128, _B, _ND, F], f32, tag="q")
                st = stpool.tile([128, _B, _ND, F], f32, tag="st")
                # cascade 1 writes p/q/st directly (duals start at 0);
                # only p's last slab is never written and must be zero.
                nc.vector.memset(p[:, :, _ND - 1, :], 0.0)

                for c in range(_CASC):
                    # z = (1-lamb)*t + lamb*sino
                    # (z-scale on DVE: first toucher of the DMA'd t tile, so
                    # the multi-queue DMA waits land here once and later DVE
                    # ops inherit them through same-engine program order.)
                    nc.scalar.mul(z[:], t[:], 1.0 - _LAMB)
                    nc.vector.tensor_tensor(z[:], z[:], ss[:], OP.add)
                    w = t  # t consumed by z; reuse as scratch
                    # ---- p chain (D axis), slabs [0, ND-1) ----
                    nc.vector.tensor_tensor(
                        w[:, :, 0 : _ND - 1, :],
                        z[:, :, 0 : _ND - 1, :],
                        z[:, :, 1:_ND, :],
                        OP.subtract,
                    )
                    if c == 0:
                        nc.vector.tensor_scalar(
                            p[:, :, 0 : _ND - 1, :],
                            w[:, :, 0 : _ND - 1, :],
                            -s0,
                            s0,
                            OP.max,
                            OP.min,
                        )
                    else:
                        nc.gpsimd.tensor_tensor(
                            p[:, :, 0 : _ND - 1, :],
                            p[:, :, 0 : _ND - 1, :],
                            w[:, :, 0 : _ND - 1, :],
                            OP.add,
                        )
                        nc.vector.tensor_scalar(
                            p[:, :, 0 : _ND - 1, :],
                            p[:, :, 0 : _ND - 1, :],
                            -s0,
                            s0,
                            OP.max,
                            OP.min,
                        )
                    for (e0, e1) in edge_ranges:
                        n = e1 - e0
                        mb = (
                            msb[:, e0:e1]
                            .unsqueeze(1)
                            .unsqueeze(3)
                            .broadcast_to([128, _B, n, F])
                        )
                        nc.vector.tensor_tensor(
                            p[:, :, e0:e1, :], p[:, :, e0:e1, :], mb, OP.mult
                        )
                    # ---- q chain (H axis) ----
                    nc.vector.tensor_tensor(
                        w[:, :, :, 0 : F - 1],
                        z[:, :, :, 0 : F - 1],
                        z[:, :, :, 1:F],
                        OP.subtract,
                    )
                    if last:
                        nc.vector.memset(w[:, :, :, F - 1 : F], 0.0)
                    if c == 0:
                        nc.vector.tensor_scalar(
                            q[:], w[:], -s1, s1, OP.max, OP.min
                        )
                    else:
                        nc.gpsimd.tensor_tensor(q[:], q[:], w[:], OP.add)
                        nc.vector.tensor_scalar(
                            q[:], q[:], -s1, s1, OP.max, OP.min
                        )
                    # ---- s~ chain (W axis via TensorE) ----
                    for d0 in range(0, _ND, _MMG):
                        g = min(_MMG, _ND - d0)
                        ps = ppool.tile([128, _B, g, F], f32, tag="ps")
                        nc.tensor.matmul(
                            ps[:],
                            matsb[:, 0, :],
                            z[:, :, d0 : d0 + g, :],
                            start=True,
                            stop=True,
                        )
                        if c == 0:
                            nc.vector.tensor_scalar(
                                st[:, :, d0 : d0 + g, :],
                                ps[:],
                                -s2,
                                s2,
                                OP.max,
                                OP.min,
                            )
                        else:
                            nc.vector.tensor_tensor(
                                st[:, :, d0 : d0 + g, :],
                                st[:, :, d0 : d0 + g, :],
                                ps[:],
                                OP.add,
                            )
                    if c > 0:
                        nc.vector.tensor_scalar(
                            st[:], st[:], -s2, s2, OP.max, OP.min
                        )
                    # ---- znew (in place) ----
                    nc.vector.tensor_scalar(z[:], z[:], -s3, s3, OP.max, OP.min)
                    # ---- t update ----
                    nc.vector.tensor_tensor(
                        t[:, :, 1:_ND, :],
                        p[:, :, 0 : _ND - 1, :],
                        p[:, :, 1:_ND, :],
                        OP.subtract,
                    )
                    nc.vector.tensor_scalar(
                        t[:, :, 0:1, :], p[:, :, 0:1, :], -1.0, None, OP.mult
                    )
                    nc.vector.tensor_tensor(t[:], t[:], z[:], OP.add)
                    nc.vector.tensor_tensor(
                        t[:, :, :, 1:F], t[:, :, :, 1:F], q[:, :, :, 0 : F - 1], OP.add
                    )
                    nc.gpsimd.tensor_tensor(t[:], t[:], q[:], OP.subtract)
                    for d0 in range(0, _ND, _MMG):
                        g = min(_MMG, _ND - d0)
                        ps = ppool.tile([128, _B, g, F], f32, tag="ps")
                        nc.tensor.matmul(
                            ps[:],
                            matsb[:, 1, :],
                            st[:, :, d0 : d0 + g, :],
                            start=True,
                            stop=True,
                        )
                        nc.vector.tensor_tensor(
                            t[:, :, d0 : d0 + g, :],
                            t[:, :, d0 : d0 + g, :],
                            ps[:],
                            OP.add,
                        )
                    for b in range(_B):
                        nc.sync.dma_start(
                            out=outs[c][:, b, :, oh : oh + og],
                            in_=t[:, b, _HALO : _HALO + _DCH, ow0:ow1],
                        )
    nc.compile()
    return nc


def _make_runner(nc, n_cores):
    """Build a reusable (cached-jit) runner for the Bass program, modeled
    on concourse.bass2jax.run_bass_via_pjrt."""
    import jax
    from jax.experimental.shard_map import shard_map
    from jax.sharding import Mesh, PartitionSpec

    from concourse import bass2jax, mybir

    bass2jax.install_neuronx_cc_hook()

    partition_name = (
        nc.partition_id_tensor.name if nc.partition_id_tensor else None
    )
    in_names, out_names, out_avals = [], [], []
    for alloc in nc.m.functions[0].allocations:
        if not isinstance(alloc, mybir.MemoryLocationSet):
            continue
        name = alloc.memorylocations[0].name
        if alloc.kind == "ExternalInput":
            if name != partition_name:
                in_names.append(name)
        elif alloc.kind == "ExternalOutput":
            shape = tuple(alloc.tensor_shape)
            dtype = mybir.dt.np(alloc.dtype)
            out_names.append(name)
            out_avals.append(jax.core.ShapedArray(shape, dtype))
    n_params = len(in_names)
    n_outs = len(out_avals)
    all_in_names = tuple(in_names + out_names + ([partition_name] if partition_name else []))
    donate = tuple(range(n_params, n_params + n_outs))

    def _body(*args):
        operands = list(args)
        if partition_name is not None:
            operands.append(bass2jax.partition_id_tensor())
        return tuple(
            bass2jax._bass_exec_p.bind(
                *operands,
                out_avals=tuple(out_avals),
                in_names=all_in_names,
                out_names=tuple(out_names),
                lowering_input_output_aliases=(),
                sim_require_finite=True,
                sim_require_nnan=True,
                nc=nc,
            )
        )

    devices = jax.devices()[:n_cores]
    assert len(devices) == n_cores
    mesh = Mesh(np.asarray(devices), ("core",))
    in_specs = (PartitionSpec("core"),) * (n_params + n_outs)
    out_specs = (PartitionSpec("core"),) * n_outs
    sharded = jax.jit(
        shard_map(
            _body, mesh=mesh, in_specs=in_specs, out_specs=out_specs, check_rep=False
        ),
        donate_argnums=donate,
        keep_unused=True,
    )

    def _concat_inputs(in_maps):
        per_core = [[np.asarray(m[name]) for name in in_names] for m in in_maps]
        return [
            np.concatenate([per_core[c][i] for c in range(n_cores)], axis=0)
            for i in range(n_params)
        ]

    def run(in_maps):
        concat_in = _concat_inputs(in_maps)
        concat_zeros = [
            np.zeros((n_cores * a.shape[0], *a.shape[1:]), a.dtype) for a in out_avals
        ]
        out_arrs = sharded(*concat_in, *concat_zeros)
        return [
            {
                name: np.asarray(out_arrs[i]).reshape(
                    n_cores, *out_avals[i].shape
                )[c]
                for i, name in enumerate(out_names)
            }
            for c in range(n_cores)
        ]

    def time_device(in_maps, reps=20):
        """Device-exec wall time with inputs pre-staged on device and
        outputs left on device (no tunnel transfer in the timed region)."""
        import time as _time

        sharded_nodonate = jax.jit(
            shard_map(
                _body,
                mesh=mesh,
                in_specs=in_specs,
                out_specs=out_specs,
                check_rep=False,
            ),
            keep_unused=True,
        )
        from jax.sharding import NamedSharding

        concat_in = _concat_inputs(in_maps)
        concat_zeros = [
            np.zeros((n_cores * a.shape[0], *a.shape[1:]), a.dtype) for a in out_avals
        ]
        shard = NamedSharding(mesh, PartitionSpec("core"))
        dev_in = [jax.device_put(x, shard) for x in concat_in]
        dev_zero = [jax.device_put(x, shard) for x in concat_zeros]
        out = sharded_nodonate(*dev_in, *dev_zero)  # warm + compile
        jax.block_until_ready(out)
        times = []
        for _ in range(reps):
            t0 = _time.perf_counter()
            out = sharded_nodonate(*dev_in, *dev_zero)
            jax.block_until_ready(out)
            times.append(_time.perf_counter() - t0)
        return times

    run.time_device = time_device
    return run


def _get_runner(sigma):
    key = tuple(float(x) for x in np.asarray(sigma).ravel())
    if key not in _RUNNER_CACHE:
        nc = _build_program(sigma)
        _RUNNER_CACHE[key] = _make_runner(nc, _NCORES)
    return _RUNNER_CACHE[key]


def _build_in_maps(image, sino):
    from concurrent.futures import ThreadPoolExecutor

    mats = _stencil_mats()

    def one_core(k):
        d0 = k * _DCH - _HALO
        # w-major chunk [W, B, ND, H], ghost slabs zero
        img_c = np.zeros((_W, _B, _ND, _H), np.float32)
        sino_c = np.zeros((_W, _B, _ND, _H), np.float32)
        lo, hi = max(0, d0), min(_D, d0 + _ND)
        img_c[:, :, lo - d0 : hi - d0, :] = image[:, lo:hi].transpose(3, 0, 1, 2)
        sino_c[:, :, lo - d0 : hi - d0, :] = sino[:, lo:hi].transpose(3, 0, 1, 2)
        gd = d0 + np.arange(_ND)
        maskp = np.broadcast_to(
            ((gd >= 0) & (gd <= _D - 2)).astype(np.float32), (128, _ND)
        ).copy()
        return {"img": img_c, "sino": sino_c, "mats": mats, "maskp": maskp}

    with ThreadPoolExecutor(max_workers=_NCORES) as ex:
        return list(ex.map(one_core, range(_NCORES)))


def _reference_numpy(image, sino, sigma, nt):
    """Slow exact fallback for unexpected inputs (e.g. nt != 0)."""
    def fwd_diff(v, ax):
        d = np.diff(v, axis=ax)
        pad = [(0, 0)] * v.ndim
        pad[ax] = (0, 1)
        return np.pad(d, pad)

    def fwd_diff_t(pp, ax):
        n = pp.shape[ax]
        pad_front = [(0, 0)] * pp.ndim
        pad_front[ax] = (1, 0)
        a = np.pad(pp, pad_front)
        a = np.take(a, range(n), axis=ax)
        pad_back = [(0, 0)] * pp.ndim
        pad_back[ax] = (0, 1)
        b = np.pad(np.take(pp, range(n - 1), axis=ax), pad_back)
        return a - b

    t = image.astype(np.float32)
    out = [t]
    p = np.zeros_like(t)
    q = np.zeros_like(t)
    s = np.zeros_like(t)
    for c in range(_CASC):
        z = t - np.float32(_LAMB) * (t - sino)
        pn = np.clip(p - fwd_diff(z, 1), -sigma[0], sigma[0])
        qn = np.clip(q - fwd_diff(z, 2), -sigma[1], sigma[1])
        sn = np.clip(s - fwd_diff(z, 3), -sigma[2], sigma[2])
        zn = np.clip(z, -sigma[3], sigma[3])
        p = pn + nt[c] * (pn - p)
        q = qn + nt[c] * (qn - q)
        s = sn + nt[c] * (sn - s)
        t = fwd_diff_t(p, 1) + fwd_diff_t(q, 2) + fwd_diff_t(s, 3) + zn
        out.append(t.astype(np.float32))
    return tuple(out)


def kernel(image, sino, sigma, nt):
    image = np.asarray(image, np.float32)
    sino = np.asarray(sino, np.float32)
    sigma = np.asarray(sigma, np.float32)
    nt = np.asarray(nt, np.float32)

    if (
        image.shape != (_B, _D, _H, _W)
        or sino.shape != (_B, _D, _H, _W)
        or np.any(nt != 0.0)
    ):
        return _reference_numpy(image, sino, sigma, nt)

    try:
        runner = _get_runner(sigma)
        in_maps = _build_in_maps(image, sino)
        try:
            results = runner(in_maps)
        except Exception:
            results = runner(in_maps)  # one retry (transient device wedge)
    except Exception:
        return _reference_numpy(image, sino, sigma, nt)

    from concurrent.futures import ThreadPoolExecutor

    def gather(c):
        # per-core [W, B, DCH, H] -> concat d -> [B, D, H, W]
        cat = np.concatenate(
            [results[k][f"out{c}"] for k in range(_NCORES)], axis=2
        )
        return np.ascontiguousarray(cat.transpose(1, 2, 3, 0))

    with ThreadPoolExecutor(max_workers=_CASC) as ex:
        full = list(ex.map(gather, range(_CASC)))
    return (image, full[0], full[1], full[2])
